# revision 1
# baseline (speedup 1.0000x reference)
"""GQA attention forward (B=1, T=2048, DIM=2048, H=16, KV=4, HD=128) on 8 trn2 cores.

Sharding: tensor-parallel over heads. Core c owns q-heads {2c, 2c+1} and kv-head
c//2 (kv work duplicated across the pair of cores sharing it). Each core:
  qT/kT/vT = projections in [hd, t] layout (f32r matmuls, N=512), RoPE on-chip
  for q/k (partition-swap via SBUF-SBUF DMA + sign-folded sin table), v
  PE-transposed to natural [t, hd] layout;
  scores S^T[k, q] = kT-block.T-contract @ qT (over hd), exp on ACT with the
  1/sqrt(hd) folded into the activation scale, causal mask via affine_select
  (fill 0 post-exp);
  A^T[hd, q] accumulates V-block.T-contract @ P^T over k-blocks in PSUM (N=512);
  denominators via ones-matrix MM -> [128, q] (broadcast across partitions);
  A^T normalized by DVE reciprocal+mul; partial out = A^T.T @ woT_c (f32r).
Host: pre-transposes x/weights, sums the 8 partial [T, DIM] outputs.
"""

import sys

if "/opt/trn_rl_repo" not in sys.path:
    sys.path.insert(0, "/opt/trn_rl_repo")

import numpy as np

T = 2048
DIM = 2048
H = 16
KV = 4
HD = 128
NCORES = 8
HPC = H // NCORES            # q heads per core = 2
SCALE = float(HD) ** -0.5
ND = DIM // 128              # dim chunks = 16
NT = T // 128                # t blocks = 16
NQC = T // 512               # q 512-chunks = 4

_CACHE = {}


def _build_nc():
    from contextlib import ExitStack

    from concourse import bacc
    import concourse.mybir as mybir
    import concourse.tile as tile
    from concourse.masks import make_identity

    f32 = mybir.dt.float32
    f32r = mybir.dt.float32r
    Exp = mybir.ActivationFunctionType.Exp

    def r(ap):
        return ap.bitcast(f32r)

    nc = bacc.Bacc("TRN2", target_bir_lowering=False, debug=False,
                   enable_asserts=False)

    xT = nc.dram_tensor("xT", [DIM, T], f32r, kind="ExternalInput").ap()
    wqT = nc.dram_tensor("wqT", [DIM, HPC * HD], f32r, kind="ExternalInput").ap()
    wkT = nc.dram_tensor("wkT", [DIM, HD], f32r, kind="ExternalInput").ap()
    wvT = nc.dram_tensor("wvT", [DIM, HD], f32r, kind="ExternalInput").ap()
    woT = nc.dram_tensor("woT", [HPC * HD, DIM], f32r, kind="ExternalInput").ap()
    cosT = nc.dram_tensor("cosT", [HD, T], f32, kind="ExternalInput").ap()
    sinT = nc.dram_tensor("sinT", [HD, T], f32, kind="ExternalInput").ap()
    out = nc.dram_tensor("out", [T, DIM], f32, kind="ExternalOutput").ap()

    with tile.TileContext(nc) as tc, ExitStack() as ctx:
        const = ctx.enter_context(tc.tile_pool(name="const", bufs=1))
        wpool = ctx.enter_context(tc.tile_pool(name="wts", bufs=1))
        qkv = ctx.enter_context(tc.tile_pool(name="qkv", bufs=1))

        ident = const.tile([128, 128], f32)
        make_identity(nc, ident)
        ones_f = const.tile([128, 128], f32)
        nc.vector.memset(ones_f, 1.0)
        ones_s = const.tile([128, 128], f32r)
        nc.scalar.copy(ones_s, ones_f)

        qT_s = qkv.tile([128, HPC * T], f32r)
        kT_s = qkv.tile([128, T], f32r)
        vT_s = qkv.tile([128, T], f32)
        v_s = qkv.tile([128, NT * HD], f32r)   # natural [t%128, hd] per t-block

        # ---- Phase 1: projections + RoPE + v-transpose, four t-quarters ----
        with tc.tile_pool(name="xp", bufs=18) as xpool, \
             tc.tile_pool(name="rope", bufs=4) as rp, \
             tc.tile_pool(name="vtp", bufs=3, space="PSUM") as vtp, \
             tc.tile_pool(name="p1ps", bufs=3, space="PSUM") as ps1:

            def load_x_quarter(tq):
                xts = []
                for d in range(ND):
                    xt = xpool.tile([128, 512], f32r, tag="xt",
                                    name=f"xt{tq}_{d}")
                    nc.sync.dma_start(
                        xt, xT[d * 128:(d + 1) * 128,
                               tq * 512:(tq + 1) * 512])
                    xts.append(xt)
                return xts

            wk_s = wpool.tile([128, ND, HD], f32r)
            nc.sync.dma_start(wk_s, wkT.rearrange("(d p) n -> p d n", p=128))
            xq = [load_x_quarter(0)]
            wq_s = wpool.tile([128, ND, HPC * HD], f32r)
            nc.sync.dma_start(wq_s, wqT.rearrange("(d p) n -> p d n", p=128))
            wv_s = wpool.tile([128, ND, HD], f32r)
            nc.sync.dma_start(wv_s, wvT.rearrange("(d p) n -> p d n", p=128))
            cos_s = const.tile([128, T], f32)
            nc.sync.dma_start(cos_s, cosT)
            sin_s = const.tile([128, T], f32)
            nc.sync.dma_start(sin_s, sinT)
            xq.append(load_x_quarter(1))

            def rope(u, c0, t0, cols=512):
                us = u[:, c0:c0 + cols]
                rot = rp.tile([128, cols], f32r, tag="rot")
                nc.sync.dma_start(rot[0:64, :], us[64:128, :])
                nc.sync.dma_start(rot[64:128, :], us[0:64, :])
                tmp = rp.tile([128, cols], f32, tag="rtmp")
                nc.vector.tensor_mul(tmp, us, cos_s[:, t0:t0 + cols])
                nc.vector.tensor_mul(rot, rot, sin_s[:, t0:t0 + cols])
                nc.vector.tensor_add(us, tmp, rot)

            def proj(acc_tag, w_ap, xts, dst, c0):
                acc = ps1.tile([128, 512], f32, tag="pps", name=acc_tag)
                for d in range(ND):
                    nc.tensor.matmul(acc, w_ap(d), r(xts[d]),
                                     start=(d == 0), stop=(d == ND - 1))
                nc.scalar.copy(dst[:, c0:c0 + 512], acc)

            for tq in range(4):
                if tq + 2 <= 3:
                    xq.append(load_x_quarter(tq + 2))
                xts = xq[tq]
                t0 = tq * 512
                proj("k", lambda d: r(wk_s[:, d, :]), xts, kT_s, t0)
                rope(kT_s, t0, t0)
                for h in range(HPC):
                    proj(f"q{h}",
                         lambda d, h=h: r(wq_s[:, d, h * HD:(h + 1) * HD]),
                         xts, qT_s, h * T + t0)
                    rope(qT_s, h * T + t0, t0)
                proj("v", lambda d: r(wv_s[:, d, :]), xts, vT_s, t0)
                for tb in range(tq * 4, tq * 4 + 4):
                    vt = vtp.tile([128, 128], f32, tag="vt")
                    nc.tensor.transpose(
                        vt, vT_s[:, tb * 128:(tb + 1) * 128], ident)
                    nc.scalar.copy(
                        v_s[:, tb * HD:(tb + 1) * HD], vt)

        # ---- Phase 2+3: attention (1-unit scores lookahead), wo interleaved ----
        apool = ctx.enter_context(tc.tile_pool(name="Apool", bufs=1))
        aT_s = [apool.tile([128, T], f32r, name=f"aT{h}") for h in range(HPC)]
        units = [(h, qc) for h in range(HPC) for qc in range(NQC)]

        with tc.tile_pool(name="sps", bufs=2, space="PSUM") as sps, \
             tc.tile_pool(name="otp", bufs=2, space="PSUM") as otp, \
             tc.tile_pool(name="dnp", bufs=2, space="PSUM") as dnp, \
             tc.tile_pool(name="pp", bufs=30) as ppool, \
             tc.tile_pool(name="rcp", bufs=2) as rpool, \
             tc.tile_pool(name="wops", bufs=2, space="PSUM") as wops, \
             tc.tile_pool(name="ost", bufs=8) as ostage:
            wo_s = wpool.tile([128, HPC, DIM], f32r)
            nc.sync.dma_start(wo_s, woT.rearrange("(h p) n -> p h n", p=128))

            def scores_burst(u):
                h, qc = units[u]
                qTh = qT_s[:, h * T:(h + 1) * T]
                nkb = 4 * qc + 4
                ptiles = []
                for kb in range(nkb):
                    s_ps = sps.tile([128, 512], f32, tag="s",
                                    name=f"s{u}_{kb}")
                    nc.tensor.matmul(
                        s_ps, r(kT_s[:, kb * 128:(kb + 1) * 128]),
                        r(qTh[:, qc * 512:(qc + 1) * 512]),
                        start=True, stop=True)
                    p_sb = ppool.tile([128, 512], f32r, tag="p",
                                      name=f"p{u}_{kb}")
                    nc.scalar.activation(p_sb, s_ps, Exp, scale=SCALE)
                    if kb >= 4 * qc:
                        nc.gpsimd.affine_select(
                            out=p_sb, in_=p_sb,
                            compare_op=mybir.AluOpType.is_ge,
                            fill=0.0, base=qc * 512 - kb * 128,
                            channel_multiplier=-1, pattern=[[1, 512]])
                    ptiles.append(p_sb)
                return ptiles

            def av_burst(u, ptiles):
                h, qc = units[u]
                nkb = 4 * qc + 4
                oT = otp.tile([128, 512], f32, tag="oT", name=f"oT{u}")
                dn = dnp.tile([128, 512], f32, tag="dn", name=f"dn{u}")
                for kb in range(nkb):
                    nc.tensor.matmul(
                        oT, r(v_s[:, kb * HD:(kb + 1) * HD]), r(ptiles[kb]),
                        start=(kb == 0), stop=(kb == nkb - 1))
                for kb in range(nkb):
                    nc.tensor.matmul(
                        dn, r(ones_s), r(ptiles[kb]),
                        start=(kb == 0), stop=(kb == nkb - 1))
                rec = rpool.tile([128, 512], f32, tag="rec")
                nc.vector.reciprocal(rec, dn)
                nc.vector.tensor_mul(
                    aT_s[h][:, qc * 512:(qc + 1) * 512], oT, rec)

            def wo_block(qc):
                for tb in range(qc * 4, qc * 4 + 4):
                    for n4 in range(4):
                        op = wops.tile([128, 512], f32, tag="op")
                        for h in range(HPC):
                            nc.tensor.matmul(
                                op, r(aT_s[h][:, tb * 128:(tb + 1) * 128]),
                                r(wo_s[:, h, n4 * 512:(n4 + 1) * 512]),
                                start=(h == 0), stop=(h == HPC - 1))
                        ob = ostage.tile([128, 512], f32, tag="ob")
                        nc.vector.tensor_copy(ob, op)
                        nc.sync.dma_start(
                            out[tb * 128:(tb + 1) * 128,
                                n4 * 512:(n4 + 1) * 512], ob)

            pending = scores_burst(0)
            for u in range(len(units)):
                nxt = scores_burst(u + 1) if u + 1 < len(units) else None
                av_burst(u, pending)
                pending = nxt
            for qc in range(NQC):
                wo_block(qc)

    nc.compile()
    return nc


def _shard_inputs(x, wq, wk, wv, wo, cos, sin):
    xTh = np.ascontiguousarray(x.reshape(T, DIM).T)
    cosTh = np.ascontiguousarray(cos.T)
    # rotate_half sign fold: out = u*cos + u_rot*sin_signed
    sinTh = np.ascontiguousarray(sin.T).copy()
    sinTh[: HD // 2, :] *= -1.0
    in_maps = []
    for c in range(NCORES):
        g = c // 2
        in_maps.append({
            "xT": xTh,
            "wqT": np.ascontiguousarray(
                wq[c * HPC * HD:(c + 1) * HPC * HD, :].T),
            "wkT": np.ascontiguousarray(wk[g * HD:(g + 1) * HD, :].T),
            "wvT": np.ascontiguousarray(wv[g * HD:(g + 1) * HD, :].T),
            "woT": np.ascontiguousarray(
                wo[:, c * HPC * HD:(c + 1) * HPC * HD].T),
            "cosT": cosTh,
            "sinT": sinTh,
        })
    return in_maps


def _get_exec():
    """Build (once) a cached jitted SPMD executable over the 8 cores.

    Mirrors bass2jax.run_bass_via_pjrt's multi-core branch, but caches the
    jitted callable so repeat kernel() calls don't re-trace/re-lower.
    """
    if "exec" in _CACHE:
        return _CACHE["exec"]

    import jax
    from jax.sharding import Mesh, PartitionSpec
    from jax.experimental.shard_map import shard_map
    from concourse import bass2jax
    import concourse.mybir as mybir

    if "nc" not in _CACHE:
        _CACHE["nc"] = _build_nc()
    nc = _CACHE["nc"]

    bass2jax.install_neuronx_cc_hook()

    part_name = (nc.partition_id_tensor.name
                 if nc.partition_id_tensor else None)
    in_names, out_names, out_avals = [], [], []
    for alloc in nc.m.functions[0].allocations:
        if not isinstance(alloc, mybir.MemoryLocationSet):
            continue
        name = alloc.memorylocations[0].name
        if alloc.kind == "ExternalInput":
            if name != part_name:
                in_names.append(name)
        elif alloc.kind == "ExternalOutput":
            out_names.append(name)
            out_avals.append(jax.core.ShapedArray(
                tuple(alloc.tensor_shape), mybir.dt.np(alloc.dtype)))

    bind_names = in_names + out_names
    if part_name is not None:
        bind_names = bind_names + [part_name]

    def _body(*args):
        operands = list(args)
        if part_name is not None:
            operands.append(bass2jax.partition_id_tensor())
        outs = bass2jax._bass_exec_p.bind(
            *operands,
            out_avals=tuple(out_avals),
            in_names=tuple(bind_names),
            out_names=tuple(out_names),
            lowering_input_output_aliases=(),
            sim_require_finite=True,
            sim_require_nnan=True,
            nc=nc,
        )
        return tuple(outs)

    devices = jax.devices()[:NCORES]
    mesh = Mesh(np.asarray(devices), ("core",))
    n_in = len(in_names)
    n_out = len(out_names)
    sharded = jax.jit(
        shard_map(
            _body, mesh=mesh,
            in_specs=(PartitionSpec("core"),) * (n_in + n_out),
            out_specs=(PartitionSpec("core"),) * n_out,
            check_rep=False,
        ),
        donate_argnums=tuple(range(n_in, n_in + n_out)),
        keep_unused=True,
    )
    _CACHE["body"] = _body
    _CACHE["exec"] = (sharded, in_names, out_names, out_avals, mesh)
    return _CACHE["exec"]


def _concat_inputs(in_maps, in_names):
    return [
        np.concatenate([in_maps[c][name] for c in range(NCORES)], axis=0)
        for name in in_names
    ]


def _zero_outs(out_avals):
    return [
        np.zeros((NCORES * a.shape[0], *a.shape[1:]), a.dtype)
        for a in out_avals
    ]


def kernel(**inputs):
    sharded, in_names, out_names, out_avals, _ = _get_exec()

    in_maps = _shard_inputs(
        np.asarray(inputs["x"], dtype=np.float32),
        np.asarray(inputs["wq"], dtype=np.float32),
        np.asarray(inputs["wk"], dtype=np.float32),
        np.asarray(inputs["wv"], dtype=np.float32),
        np.asarray(inputs["wo"], dtype=np.float32),
        np.asarray(inputs["cos"], dtype=np.float32),
        np.asarray(inputs["sin"], dtype=np.float32),
    )
    concat_in = _concat_inputs(in_maps, in_names)
    out_arrs = sharded(*concat_in, *_zero_outs(out_avals))

    full = np.asarray(out_arrs[out_names.index("out")])
    acc = full.reshape(NCORES, T, DIM).astype(np.float32).sum(axis=0)
    return acc.reshape(1, T, DIM)



# revision 16
# speedup vs baseline: 397.8999x; 397.8999x over previous
"""GQA attention forward (B=1, T=2048, DIM=2048, H=16, KV=4, HD=128) on 8 trn2 cores.

Sharding: tensor-parallel over heads. Core c owns q-heads {2c, 2c+1} and kv-head
c//2 (kv work duplicated across the pair of cores sharing it).

Kernel structure (per core), quarter-major pipeline over four 512-token chunks:
  per quarter tq: qkv projections (fp16 matmuls, f32 PSUM), RoPE on q/k with the
  rotate-half partition swap done as a PE matmul against a host-supplied
  permutation matrix (sign folded into the sin table), v PE-transposed to
  natural [t, hd] fp16 layout; then per q-head: causal scores S^T[k, q] (f32r,
  diagonal blocks trimmed to the valid q-range, min N=256), causal mask applied
  as a matmul-accumulated additive bias (only 4 distinct mask matrices exist,
  host-supplied), exp on ACT with 1/sqrt(hd) folded into the activation scale
  (fp16 out), softmax denominators via ones-matmul accumulation in PSUM, A^T
  accumulated in PSUM over k-blocks, normalized by DVE reciprocal+mul into fp16
  aT; wo matmuls (fp16) interleaved one quarter behind so output DMA streams
  during attention.

All DMAs use >=2KB-per-partition lines: weights/masks are pre-rearranged on the
host into their SBUF layouts (contiguous loads), x is loaded as [128,1024] fp16
half-tiles, out is staged and written as [128,1024] fp16 pairs.

Host: sums the 8 partial [T, DIM] fp16 outputs in f32.

_build_nc(nrepeat=N) wraps the whole body in an on-device For_i loop (used by
test.py to measure per-iteration HW exec time without dispatch overhead).
"""

import sys

if "/opt/trn_rl_repo" not in sys.path:
    sys.path.insert(0, "/opt/trn_rl_repo")

import numpy as np

T = 2048
DIM = 2048
H = 16
KV = 4
HD = 128
NCORES = 8
HPC = H // NCORES            # q heads per core = 2
SCALE = float(HD) ** -0.5
ND = DIM // 128              # dim chunks = 16
NT = T // 128                # t blocks = 16
NQC = T // 512               # q 512-chunks = 4

# causal diagonal-block trim: for diag offset j, start q at QOFF[j] (N>=256)
QOFF = [0, 128, 256, 256]

_CACHE = {}


def _build_nc(nrepeat=1):
    from contextlib import ExitStack, nullcontext

    from concourse import bacc
    import concourse.mybir as mybir
    import concourse.tile as tile
    from concourse.masks import make_identity

    f32 = mybir.dt.float32
    f32r = mybir.dt.float32r
    f16 = mybir.dt.float16
    Exp = mybir.ActivationFunctionType.Exp

    nc = bacc.Bacc("TRN2", target_bir_lowering=False, debug=False,
                   enable_asserts=False)

    # all weight-like inputs arrive pre-rearranged into their SBUF layouts
    xT = nc.dram_tensor("xT", [DIM, T], f16, kind="ExternalInput").ap()
    wqT = nc.dram_tensor("wqT", [128, ND * HPC * HD], f16,
                         kind="ExternalInput").ap()
    wkT = nc.dram_tensor("wkT", [128, ND * HD], f16, kind="ExternalInput").ap()
    wvT = nc.dram_tensor("wvT", [128, ND * HD], f16, kind="ExternalInput").ap()
    woT = nc.dram_tensor("woT", [128, HPC * DIM], f16,
                         kind="ExternalInput").ap()
    cosT = nc.dram_tensor("cosT", [HD, T], f16, kind="ExternalInput").ap()
    sinT = nc.dram_tensor("sinT", [HD, T], f16, kind="ExternalInput").ap()
    swpT = nc.dram_tensor("swpT", [128, 128], f32r, kind="ExternalInput").ap()
    idT = nc.dram_tensor("idT", [128, 128], f32r, kind="ExternalInput").ap()
    mskT = nc.dram_tensor("mskT", [128, 4 * 512], f32r,
                          kind="ExternalInput").ap()
    out = nc.dram_tensor("out", [T, DIM], f16, kind="ExternalOutput").ap()

    with tile.TileContext(nc) as tc, \
         (tc.For_i(0, nrepeat) if nrepeat > 1 else nullcontext()), \
         ExitStack() as ctx:
        const = ctx.enter_context(tc.tile_pool(name="const", bufs=1))
        wpool = ctx.enter_context(tc.tile_pool(name="wts", bufs=1))
        qkv = ctx.enter_context(tc.tile_pool(name="qkv", bufs=1))
        apool = ctx.enter_context(tc.tile_pool(name="Apool", bufs=1))
        xpool = ctx.enter_context(tc.tile_pool(name="xp", bufs=16))
        rp = ctx.enter_context(tc.tile_pool(name="rope", bufs=2))
        ppool = ctx.enter_context(tc.tile_pool(name="pp", bufs=28))
        rpool = ctx.enter_context(tc.tile_pool(name="rcp", bufs=2))
        ostage = ctx.enter_context(tc.tile_pool(name="ost", bufs=4))
        ps1 = ctx.enter_context(tc.tile_pool(name="p1ps", bufs=3, space="PSUM"))
        sps = ctx.enter_context(tc.tile_pool(name="sps", bufs=2, space="PSUM"))
        otp = ctx.enter_context(tc.tile_pool(name="otp", bufs=1, space="PSUM"))
        wops = ctx.enter_context(tc.tile_pool(name="wops", bufs=2, space="PSUM"))

        ident = const.tile([128, 128], f16)
        make_identity(nc, ident)
        ones16 = const.tile([128, 128], f16)
        nc.vector.memset(ones16, 1.0)

        qT_s = qkv.tile([128, HPC * T], f32r)   # rope'd q, [hd, t] per head
        kT_s = qkv.tile([128, T], f32r)         # rope'd k, [hd, t]
        vT_s = qkv.tile([128, T], f16)          # v, [hd, t]
        v_s = qkv.tile([128, NT * HD], f16)     # v natural [t%128, hd] per blk
        aT_s = [apool.tile([128, T], f16, name=f"aT{h}") for h in range(HPC)]

        def load_x_half(i):
            xts = []
            for d in range(ND):
                xt = xpool.tile([128, 1024], f16, tag="xt",
                                name=f"xt{i}_{d}")
                nc.sync.dma_start(
                    xt, xT[d * 128:(d + 1) * 128, i * 1024:(i + 1) * 1024])
                xts.append(xt)
            return xts

        def load_x_tiles(i, lo, hi, xts):
            for d in range(lo, hi):
                xt = xpool.tile([128, 1024], f16, tag="xt",
                                name=f"xt{i}_{d}")
                nc.sync.dma_start(
                    xt, xT[d * 128:(d + 1) * 128, i * 1024:(i + 1) * 1024])
                xts.append(xt)

        wk_s = wpool.tile([128, ND * HD], f16)
        nc.sync.dma_start(wk_s, wkT)
        xh0 = []
        load_x_tiles(0, 0, 8, xh0)
        wq_s = wpool.tile([128, ND * HPC * HD], f16)
        nc.sync.dma_start(wq_s, wqT)
        load_x_tiles(0, 8, 12, xh0)
        cos_s = const.tile([128, T], f16)
        nc.sync.dma_start(cos_s, cosT)
        sin_s = const.tile([128, T], f16)
        nc.sync.dma_start(sin_s, sinT)
        swp_s = const.tile([128, 128], f32r)
        nc.sync.dma_start(swp_s, swpT)
        load_x_tiles(0, 12, 16, xh0)
        wv_s = wpool.tile([128, ND * HD], f16)
        nc.sync.dma_start(wv_s, wvT)
        id_s = const.tile([128, 128], f32r)
        nc.sync.dma_start(id_s, idT)
        msk_s = const.tile([128, 4 * 512], f32r)
        nc.sync.dma_start(msk_s, mskT)
        wo_s = wpool.tile([128, HPC * DIM], f16)
        nc.sync.dma_start(wo_s, woT)
        xh1 = load_x_half(1)
        xh = [xh0, xh1]

        def xts_for(tq):
            half = xh[tq // 2]
            c0 = (tq % 2) * 512
            return [t[:, c0:c0 + 512] for t in half]

        def rope(u, c0, t0, cols=512):
            # u[:, c0:c0+cols] <- u*cos + rotate_half(u)*sin  (sin sign-folded)
            us = u[:, c0:c0 + cols]
            cs = cos_s[:, t0:t0 + cols]
            ss = sin_s[:, t0:t0 + cols]
            tmp = rp.tile([128, cols], f32, tag="rtmp")
            rot_ps = ps1.tile([128, cols], f32, tag="pps", name="rotp")
            # partition swap as a PE matmul against a permutation matrix
            nc.tensor.matmul(rot_ps, swp_s, us, start=True, stop=True)
            rot = rp.tile([128, cols], f32, tag="rot")
            nc.vector.tensor_mul(tmp, us, cs)
            nc.vector.tensor_mul(rot, rot_ps, ss)
            nc.vector.tensor_add(us, tmp, rot)

        def proj(acc_tag, w_ap, xts, dst, c0):
            acc = ps1.tile([128, 512], f32, tag="pps", name=acc_tag)
            for d in range(ND):
                nc.tensor.matmul(acc, w_ap(d), xts[d],
                                 start=(d == 0), stop=(d == ND - 1))
            nc.scalar.copy(dst[:, c0:c0 + 512], acc)

        def quarter_proj(tq):
            xts = xts_for(tq)
            t0 = tq * 512
            proj(f"k{tq}", lambda d: wk_s[:, d * HD:(d + 1) * HD],
                 xts, kT_s, t0)
            proj(f"q0_{tq}",
                 lambda d: wq_s[:, d * HPC * HD:d * HPC * HD + HD],
                 xts, qT_s, t0)
            rope(kT_s, t0, t0)
            proj(f"q1_{tq}",
                 lambda d: wq_s[:, d * HPC * HD + HD:(d + 1) * HPC * HD],
                 xts, qT_s, T + t0)
            rope(qT_s, t0, t0)
            proj(f"v{tq}", lambda d: wv_s[:, d * HD:(d + 1) * HD],
                 xts, vT_s, t0)
            rope(qT_s, T + t0, t0)
            for tb in range(tq * 4, tq * 4 + 4):
                vt = ps1.tile([128, 128], f16, tag="pps", name=f"vt{tb}")
                nc.tensor.transpose(
                    vt, vT_s[:, tb * 128:(tb + 1) * 128], ident)
                nc.scalar.copy(v_s[:, tb * HD:(tb + 1) * HD], vt)

        def scores_burst(h, qc):
            qTh = qT_s[:, h * T:(h + 1) * T]
            nkb = 4 * qc + 4
            ptiles = []
            for kb in range(nkb):
                j = kb - 4 * qc
                qo = QOFF[j] if j >= 0 else 0
                n = 512 - qo
                s_ps = sps.tile([128, n], f32, tag="s", name=f"s{h}{qc}_{kb}")
                nc.tensor.matmul(
                    s_ps, kT_s[:, kb * 128:(kb + 1) * 128],
                    qTh[:, qc * 512 + qo:(qc + 1) * 512],
                    start=True, stop=(j < 0))
                if j >= 0:
                    # causal mask as an accumulated additive bias matmul
                    nc.tensor.matmul(
                        s_ps, id_s, msk_s[:, j * 512:j * 512 + n],
                        start=False, stop=True)
                p_sb = ppool.tile([128, n], f16, tag="p", name=f"p{h}{qc}_{kb}")
                nc.scalar.activation(p_sb, s_ps, Exp, scale=SCALE)
                ptiles.append((p_sb, qo))
            return ptiles

        def av_burst(h, qc, ptiles):
            nkb = 4 * qc + 4
            oT = otp.tile([128, 512], f32, tag="oT", name=f"oT{h}_{qc}")
            # dn shares ps1's "pps" slots (proj accs and dn alternate in time)
            dn = ps1.tile([128, 512], f32, tag="pps", name=f"dn{h}_{qc}")
            for kb in range(nkb):
                p_sb, qo = ptiles[kb]
                nc.tensor.matmul(
                    oT[:, qo:], v_s[:, kb * HD:(kb + 1) * HD], p_sb,
                    start=(kb == 0), stop=(kb == nkb - 1))
            for kb in range(nkb):
                p_sb, qo = ptiles[kb]
                nc.tensor.matmul(
                    dn[:, qo:], ones16, p_sb,
                    start=(kb == 0), stop=(kb == nkb - 1))
            rec = rpool.tile([128, 512], f32, tag="rec")
            nc.vector.reciprocal(rec, dn)
            nc.vector.tensor_mul(
                aT_s[h][:, qc * 512:(qc + 1) * 512], oT, rec)

        def wo_block(qc):
            for tb in range(qc * 4, qc * 4 + 4):
                for n2 in range(2):
                    ops = []
                    for k in range(2):
                        op = wops.tile([128, 512], f32, tag="op",
                                       name=f"op{tb}_{n2}_{k}")
                        for h in range(HPC):
                            nc.tensor.matmul(
                                op, aT_s[h][:, tb * 128:(tb + 1) * 128],
                                wo_s[:, h * DIM + (n2 * 2 + k) * 512:
                                     h * DIM + (n2 * 2 + k + 1) * 512],
                                start=(h == 0), stop=(h == HPC - 1))
                        ops.append(op)
                    ob = ostage.tile([128, 1024], f16, tag="ob")
                    nc.scalar.copy(ob[:, 0:512], ops[0])
                    nc.vector.tensor_copy(ob[:, 512:1024], ops[1])
                    nc.sync.dma_start(
                        out[tb * 128:(tb + 1) * 128,
                            n2 * 1024:(n2 + 1) * 1024], ob)

        for tq in range(4):
            quarter_proj(tq)
            if tq > 0:
                wo_block(tq - 1)
            p0 = scores_burst(0, tq)
            p1 = scores_burst(1, tq)
            av_burst(0, tq, p0)
            av_burst(1, tq, p1)
        wo_block(3)

    nc.compile()
    return nc


def _mask_tables():
    # M_j[p, qf] = 0 where (QOFF[j] + qf) - 128*j - p >= 0 else -30000
    msk = np.zeros((4, 128, 512), np.float32)
    p = np.arange(128)[:, None]
    qf = np.arange(512)[None, :]
    for j in range(4):
        cond = (QOFF[j] + qf - 128 * j - p) >= 0
        msk[j] = np.where(cond, 0.0, -30000.0)
    # [p, j*512+qf] layout
    mskh = np.ascontiguousarray(msk.transpose(1, 0, 2).reshape(128, 4 * 512))
    swp = np.zeros((128, 128), np.float32)
    # rot = swp.T @ u : rot[i] = u[i+64] (i<64), rot[i] = u[i-64] (i>=64)
    for i in range(64):
        swp[i + 64, i] = 1.0
        swp[i, i + 64] = 1.0
    return mskh, swp


def _sbufify(w):
    """[rows=outdim, DIM] weight -> pre-rearranged [128, ND*rows] fp16 with the
    d-chunk-major SBUF layout: dst[p, d*rows + n] = w[n, d*128 + p]."""
    rows = w.shape[0]
    a = np.ascontiguousarray(
        w.T.reshape(ND, 128, rows).transpose(1, 0, 2).reshape(128, ND * rows))
    return a.astype(np.float16)


def _shard_inputs(x, wq, wk, wv, wo, cos, sin):
    mskh, swph = _mask_tables()
    idh = np.eye(128, dtype=np.float32)
    xTh = np.ascontiguousarray(x.reshape(T, DIM).T.astype(np.float16))
    cosTh = np.ascontiguousarray(cos.T)
    # rotate_half sign fold: out = u*cos + u_rot*sin_signed
    sinTh = np.ascontiguousarray(sin.T).copy()
    sinTh[: HD // 2, :] *= -1.0
    in_maps = []
    for c in range(NCORES):
        g = c // 2
        # wo slice: [DIM, HPC*HD] -> [128, HPC*DIM] with [p, h*DIM+n] layout
        woc = wo[:, c * HPC * HD:(c + 1) * HPC * HD]  # [DIM, 256]
        woh = np.ascontiguousarray(
            woc.T.reshape(HPC, 128, DIM).transpose(1, 0, 2)
            .reshape(128, HPC * DIM)).astype(np.float16)
        in_maps.append({
            "xT": xTh,
            "wqT": _sbufify(wq[c * HPC * HD:(c + 1) * HPC * HD, :]),
            "wkT": _sbufify(wk[g * HD:(g + 1) * HD, :]),
            "wvT": _sbufify(wv[g * HD:(g + 1) * HD, :]),
            "woT": woh,
            "cosT": cosTh.astype(np.float16),
            "sinT": sinTh.astype(np.float16),
            "swpT": swph,
            "idT": idh,
            "mskT": mskh,
        })
    return in_maps


def _get_exec(nrepeat=1):
    """Build (once) a cached jitted SPMD executable over the 8 cores."""
    key = ("exec", nrepeat)
    if key in _CACHE:
        return _CACHE[key]

    import jax
    from jax.sharding import Mesh, PartitionSpec
    from jax.experimental.shard_map import shard_map
    from concourse import bass2jax
    import concourse.mybir as mybir

    nc = _build_nc(nrepeat=nrepeat)

    bass2jax.install_neuronx_cc_hook()

    part_name = (nc.partition_id_tensor.name
                 if nc.partition_id_tensor else None)
    in_names, out_names, out_avals = [], [], []
    for alloc in nc.m.functions[0].allocations:
        if not isinstance(alloc, mybir.MemoryLocationSet):
            continue
        name = alloc.memorylocations[0].name
        if alloc.kind == "ExternalInput":
            if name != part_name:
                in_names.append(name)
        elif alloc.kind == "ExternalOutput":
            out_names.append(name)
            out_avals.append(jax.core.ShapedArray(
                tuple(alloc.tensor_shape), mybir.dt.np(alloc.dtype)))

    bind_names = in_names + out_names
    if part_name is not None:
        bind_names = bind_names + [part_name]

    def _body(*args):
        operands = list(args)
        if part_name is not None:
            operands.append(bass2jax.partition_id_tensor())
        outs = bass2jax._bass_exec_p.bind(
            *operands,
            out_avals=tuple(out_avals),
            in_names=tuple(bind_names),
            out_names=tuple(out_names),
            lowering_input_output_aliases=(),
            sim_require_finite=True,
            sim_require_nnan=True,
            nc=nc,
        )
        return tuple(outs)

    devices = jax.devices()[:NCORES]
    mesh = Mesh(np.asarray(devices), ("core",))
    n_in = len(in_names)
    n_out = len(out_names)
    sharded = jax.jit(
        shard_map(
            _body, mesh=mesh,
            in_specs=(PartitionSpec("core"),) * (n_in + n_out),
            out_specs=(PartitionSpec("core"),) * n_out,
            check_rep=False,
        ),
        keep_unused=True,
    )
    _CACHE[key] = (sharded, in_names, out_names, out_avals, mesh)
    return _CACHE[key]


def _concat_inputs(in_maps, in_names):
    return [
        np.concatenate([in_maps[c][name] for c in range(NCORES)], axis=0)
        for name in in_names
    ]


def _zero_outs(out_avals):
    return [
        np.zeros((NCORES * a.shape[0], *a.shape[1:]), a.dtype)
        for a in out_avals
    ]


def kernel(**inputs):
    sharded, in_names, out_names, out_avals, _ = _get_exec()

    in_maps = _shard_inputs(
        np.asarray(inputs["x"], dtype=np.float32),
        np.asarray(inputs["wq"], dtype=np.float32),
        np.asarray(inputs["wk"], dtype=np.float32),
        np.asarray(inputs["wv"], dtype=np.float32),
        np.asarray(inputs["wo"], dtype=np.float32),
        np.asarray(inputs["cos"], dtype=np.float32),
        np.asarray(inputs["sin"], dtype=np.float32),
    )
    concat_in = _concat_inputs(in_maps, in_names)
    out_arrs = sharded(*concat_in, *_zero_outs(out_avals))

    full = np.asarray(out_arrs[out_names.index("out")])
    acc = full.reshape(NCORES, T, DIM).astype(np.float32).sum(axis=0)
    return acc.reshape(1, T, DIM)


# revision 19
# speedup vs baseline: 400.1256x; 1.0056x over previous
"""GQA attention forward (B=1, T=2048, DIM=2048, H=16, KV=4, HD=128) on 8 trn2 cores.

Sharding: tensor-parallel over heads. Core c owns q-heads {2c, 2c+1} and kv-head
c//2 (kv work duplicated across the pair of cores sharing it).

Kernel structure (per core), quarter-major pipeline over four 512-token chunks:
  per quarter tq: qkv projections (fp16 matmuls, f32 PSUM), RoPE on q/k with the
  rotate-half partition swap done as a PE matmul against a host-supplied
  permutation matrix (sign folded into the sin table), v PE-transposed to
  natural [t, hd] fp16 layout; then per q-head: causal scores S^T[k, q] (f32r,
  diagonal blocks trimmed to the valid q-range, min N=256), causal mask applied
  as a matmul-accumulated additive bias (only 4 distinct mask matrices exist,
  host-supplied), exp on ACT with 1/sqrt(hd) folded into the activation scale
  (fp16 out), softmax denominators via ones-matmul accumulation in PSUM, A^T
  accumulated in PSUM over k-blocks, normalized by DVE reciprocal+mul into fp16
  aT; wo matmuls (fp16) interleaved one quarter behind so output DMA streams
  during attention.

All DMAs use >=2KB-per-partition lines: weights/masks are pre-rearranged on the
host into their SBUF layouts (contiguous loads), x is loaded as [128,1024] fp16
half-tiles, out is staged and written as [128,1024] fp16 pairs.

Host: sums the 8 partial [T, DIM] fp16 outputs in f32.

_build_nc(nrepeat=N) wraps the whole body in an on-device For_i loop (used by
test.py to measure per-iteration HW exec time without dispatch overhead).
"""

import sys

if "/opt/trn_rl_repo" not in sys.path:
    sys.path.insert(0, "/opt/trn_rl_repo")

import numpy as np

T = 2048
DIM = 2048
H = 16
KV = 4
HD = 128
NCORES = 8
HPC = H // NCORES            # q heads per core = 2
SCALE = float(HD) ** -0.5
ND = DIM // 128              # dim chunks = 16
NT = T // 128                # t blocks = 16
NQC = T // 512               # q 512-chunks = 4

# causal diagonal-block trim: for diag offset j, start q at QOFF[j] (N>=256)
QOFF = [0, 128, 256, 256]

_CACHE = {}


def _build_nc(nrepeat=1):
    from contextlib import ExitStack, nullcontext

    from concourse import bacc
    import concourse.mybir as mybir
    import concourse.tile as tile
    from concourse.masks import make_identity

    f32 = mybir.dt.float32
    f32r = mybir.dt.float32r
    f16 = mybir.dt.float16
    Exp = mybir.ActivationFunctionType.Exp

    nc = bacc.Bacc("TRN2", target_bir_lowering=False, debug=False,
                   enable_asserts=False)

    # all weight-like inputs arrive pre-rearranged into their SBUF layouts
    xT = nc.dram_tensor("xT", [DIM, T], f16, kind="ExternalInput").ap()
    wqT = nc.dram_tensor("wqT", [128, ND * HPC * HD], f16,
                         kind="ExternalInput").ap()
    wkT = nc.dram_tensor("wkT", [128, ND * HD], f16, kind="ExternalInput").ap()
    wvT = nc.dram_tensor("wvT", [128, ND * HD], f16, kind="ExternalInput").ap()
    woT = nc.dram_tensor("woT", [128, HPC * DIM], f16,
                         kind="ExternalInput").ap()
    cosT = nc.dram_tensor("cosT", [HD, T], f16, kind="ExternalInput").ap()
    sinT = nc.dram_tensor("sinT", [HD, T], f16, kind="ExternalInput").ap()
    swpT = nc.dram_tensor("swpT", [128, 128], f32r, kind="ExternalInput").ap()
    idT = nc.dram_tensor("idT", [128, 128], f32r, kind="ExternalInput").ap()
    mskT = nc.dram_tensor("mskT", [128, 4 * 512], f32r,
                          kind="ExternalInput").ap()
    out = nc.dram_tensor("out", [T, DIM], f16, kind="ExternalOutput").ap()

    with tile.TileContext(nc) as tc, \
         (tc.For_i(0, nrepeat) if nrepeat > 1 else nullcontext()), \
         ExitStack() as ctx:
        const = ctx.enter_context(tc.tile_pool(name="const", bufs=1))
        wpool = ctx.enter_context(tc.tile_pool(name="wts", bufs=1))
        qkv = ctx.enter_context(tc.tile_pool(name="qkv", bufs=1))
        apool = ctx.enter_context(tc.tile_pool(name="Apool", bufs=1))
        xpool = ctx.enter_context(tc.tile_pool(name="xp", bufs=16))
        rp = ctx.enter_context(tc.tile_pool(name="rope", bufs=2))
        ppool = ctx.enter_context(tc.tile_pool(name="pp", bufs=32))
        rpool = ctx.enter_context(tc.tile_pool(name="rcp", bufs=2))
        ostage = ctx.enter_context(tc.tile_pool(name="ost", bufs=4))
        ps1 = ctx.enter_context(tc.tile_pool(name="p1ps", bufs=3, space="PSUM"))
        sps = ctx.enter_context(tc.tile_pool(name="sps", bufs=2, space="PSUM"))
        otp = ctx.enter_context(tc.tile_pool(name="otp", bufs=1, space="PSUM"))
        wops = ctx.enter_context(tc.tile_pool(name="wops", bufs=2, space="PSUM"))

        ident = const.tile([128, 128], f16)
        make_identity(nc, ident)
        ones16 = const.tile([128, 128], f16)
        nc.vector.memset(ones16, 1.0)

        qT_s = qkv.tile([128, HPC * T], f32r)   # rope'd q, [hd, t] per head
        kT_s = qkv.tile([128, T], f32r)         # rope'd k, [hd, t]
        vT_s = qkv.tile([128, T], f16)          # v, [hd, t]
        v_s = qkv.tile([128, NT * HD], f16)     # v natural [t%128, hd] per blk
        aT_s = [apool.tile([128, T], f16, name=f"aT{h}") for h in range(HPC)]

        def load_x_tiles(c0, cols, lo, hi, xts, tag):
            nb = 32 if tag == "xq" else 16
            for d in range(lo, hi):
                xt = xpool.tile([128, cols], f16, tag=tag, bufs=nb,
                                name=f"xt{c0}_{d}")
                nc.sync.dma_start(
                    xt, xT[d * 128:(d + 1) * 128, c0:c0 + cols])
                xts.append(xt)

        # quarter 0 narrow (startup-critical), middle half wide, quarter 3
        # narrow — minimizes bytes gating the first projections
        wk_s = wpool.tile([128, ND * HD], f16)
        nc.sync.dma_start(wk_s, wkT)
        xq0 = []
        load_x_tiles(0, 512, 0, 8, xq0, "xq")
        wq_s = wpool.tile([128, ND * HPC * HD], f16)
        nc.sync.dma_start(wq_s, wqT)
        load_x_tiles(0, 512, 8, 12, xq0, "xq")
        cos_s = const.tile([128, T], f16)
        nc.sync.dma_start(cos_s, cosT)
        sin_s = const.tile([128, T], f16)
        nc.sync.dma_start(sin_s, sinT)
        swp_s = const.tile([128, 128], f32r)
        nc.sync.dma_start(swp_s, swpT)
        load_x_tiles(0, 512, 12, 16, xq0, "xq")
        wv_s = wpool.tile([128, ND * HD], f16)
        nc.sync.dma_start(wv_s, wvT)
        id_s = const.tile([128, 128], f32r)
        nc.sync.dma_start(id_s, idT)
        msk_s = const.tile([128, 4 * 512], f32r)
        nc.sync.dma_start(msk_s, mskT)
        xmid = []
        load_x_tiles(512, 1024, 0, 16, xmid, "xt")
        wo_s = wpool.tile([128, HPC * DIM], f16)
        nc.sync.dma_start(wo_s, woT)
        xq3 = []
        load_x_tiles(1536, 512, 0, 16, xq3, "xq")

        def xts_for(tq):
            if tq == 0:
                return xq0
            if tq == 3:
                return xq3
            c0 = (tq - 1) * 512
            return [t[:, c0:c0 + 512] for t in xmid]

        def rope(u, c0, t0, cols=512):
            # u[:, c0:c0+cols] <- u*cos + rotate_half(u)*sin  (sin sign-folded)
            us = u[:, c0:c0 + cols]
            cs = cos_s[:, t0:t0 + cols]
            ss = sin_s[:, t0:t0 + cols]
            tmp = rp.tile([128, cols], f32, tag="rtmp")
            rot_ps = ps1.tile([128, cols], f32, tag="pps", name="rotp")
            # partition swap as a PE matmul against a permutation matrix
            nc.tensor.matmul(rot_ps, swp_s, us, start=True, stop=True)
            rot = rp.tile([128, cols], f32, tag="rot")
            nc.vector.tensor_mul(tmp, us, cs)
            nc.vector.tensor_mul(rot, rot_ps, ss)
            nc.vector.tensor_add(us, tmp, rot)

        def proj(acc_tag, w_ap, xts, dst, c0):
            acc = ps1.tile([128, 512], f32, tag="pps", name=acc_tag)
            for d in range(ND):
                nc.tensor.matmul(acc, w_ap(d), xts[d],
                                 start=(d == 0), stop=(d == ND - 1))
            nc.scalar.copy(dst[:, c0:c0 + 512], acc)

        def quarter_proj(tq):
            xts = xts_for(tq)
            t0 = tq * 512
            proj(f"k{tq}", lambda d: wk_s[:, d * HD:(d + 1) * HD],
                 xts, kT_s, t0)
            proj(f"q0_{tq}",
                 lambda d: wq_s[:, d * HPC * HD:d * HPC * HD + HD],
                 xts, qT_s, t0)
            rope(kT_s, t0, t0)
            proj(f"q1_{tq}",
                 lambda d: wq_s[:, d * HPC * HD + HD:(d + 1) * HPC * HD],
                 xts, qT_s, T + t0)
            rope(qT_s, t0, t0)
            proj(f"v{tq}", lambda d: wv_s[:, d * HD:(d + 1) * HD],
                 xts, vT_s, t0)
            rope(qT_s, T + t0, t0)
            for tb in range(tq * 4, tq * 4 + 4):
                vt = ps1.tile([128, 128], f16, tag="pps", name=f"vt{tb}")
                nc.tensor.transpose(
                    vt, vT_s[:, tb * 128:(tb + 1) * 128], ident)
                nc.scalar.copy(v_s[:, tb * HD:(tb + 1) * HD], vt)

        def scores_burst(h, qc):
            qTh = qT_s[:, h * T:(h + 1) * T]
            nkb = 4 * qc + 4
            ptiles = []
            for kb in range(nkb):
                j = kb - 4 * qc
                qo = QOFF[j] if j >= 0 else 0
                n = 512 - qo
                s_ps = sps.tile([128, n], f32, tag="s", name=f"s{h}{qc}_{kb}")
                nc.tensor.matmul(
                    s_ps, kT_s[:, kb * 128:(kb + 1) * 128],
                    qTh[:, qc * 512 + qo:(qc + 1) * 512],
                    start=True, stop=(j < 0))
                if j >= 0:
                    # causal mask as an accumulated additive bias matmul
                    nc.tensor.matmul(
                        s_ps, id_s, msk_s[:, j * 512:j * 512 + n],
                        start=False, stop=True)
                p_sb = ppool.tile([128, n], f16, tag="p", name=f"p{h}{qc}_{kb}")
                nc.scalar.activation(p_sb, s_ps, Exp, scale=SCALE)
                ptiles.append((p_sb, qo))
            return ptiles

        def av_burst(h, qc, ptiles):
            nkb = 4 * qc + 4
            oT = otp.tile([128, 512], f32, tag="oT", name=f"oT{h}_{qc}")
            # dn shares ps1's "pps" slots (proj accs and dn alternate in time)
            dn = ps1.tile([128, 512], f32, tag="pps", name=f"dn{h}_{qc}")
            for kb in range(nkb):
                p_sb, qo = ptiles[kb]
                nc.tensor.matmul(
                    oT[:, qo:], v_s[:, kb * HD:(kb + 1) * HD], p_sb,
                    start=(kb == 0), stop=(kb == nkb - 1))
            for kb in range(nkb):
                p_sb, qo = ptiles[kb]
                nc.tensor.matmul(
                    dn[:, qo:], ones16, p_sb,
                    start=(kb == 0), stop=(kb == nkb - 1))
            rec = rpool.tile([128, 512], f32, tag="rec")
            nc.vector.reciprocal(rec, dn)
            nc.vector.tensor_mul(
                aT_s[h][:, qc * 512:(qc + 1) * 512], oT, rec)

        def wo_block(qc):
            for tb in range(qc * 4, qc * 4 + 4):
                for n2 in range(2):
                    ops = []
                    for k in range(2):
                        op = wops.tile([128, 512], f32, tag="op",
                                       name=f"op{tb}_{n2}_{k}")
                        for h in range(HPC):
                            nc.tensor.matmul(
                                op, aT_s[h][:, tb * 128:(tb + 1) * 128],
                                wo_s[:, h * DIM + (n2 * 2 + k) * 512:
                                     h * DIM + (n2 * 2 + k + 1) * 512],
                                start=(h == 0), stop=(h == HPC - 1))
                        ops.append(op)
                    ob = ostage.tile([128, 1024], f16, tag="ob")
                    nc.scalar.copy(ob[:, 0:512], ops[0])
                    nc.vector.tensor_copy(ob[:, 512:1024], ops[1])
                    nc.sync.dma_start(
                        out[tb * 128:(tb + 1) * 128,
                            n2 * 1024:(n2 + 1) * 1024], ob)

        for tq in range(4):
            quarter_proj(tq)
            if tq > 0:
                wo_block(tq - 1)
            p0 = scores_burst(0, tq)
            p1 = scores_burst(1, tq)
            av_burst(0, tq, p0)
            av_burst(1, tq, p1)
        wo_block(3)

    nc.compile()
    return nc


def _mask_tables():
    # M_j[p, qf] = 0 where (QOFF[j] + qf) - 128*j - p >= 0 else -30000
    msk = np.zeros((4, 128, 512), np.float32)
    p = np.arange(128)[:, None]
    qf = np.arange(512)[None, :]
    for j in range(4):
        cond = (QOFF[j] + qf - 128 * j - p) >= 0
        msk[j] = np.where(cond, 0.0, -30000.0)
    # [p, j*512+qf] layout
    mskh = np.ascontiguousarray(msk.transpose(1, 0, 2).reshape(128, 4 * 512))
    swp = np.zeros((128, 128), np.float32)
    # rot = swp.T @ u : rot[i] = u[i+64] (i<64), rot[i] = u[i-64] (i>=64)
    for i in range(64):
        swp[i + 64, i] = 1.0
        swp[i, i + 64] = 1.0
    return mskh, swp


def _sbufify(w):
    """[rows=outdim, DIM] weight -> pre-rearranged [128, ND*rows] fp16 with the
    d-chunk-major SBUF layout: dst[p, d*rows + n] = w[n, d*128 + p]."""
    rows = w.shape[0]
    a = np.ascontiguousarray(
        w.T.reshape(ND, 128, rows).transpose(1, 0, 2).reshape(128, ND * rows))
    return a.astype(np.float16)


def _shard_inputs(x, wq, wk, wv, wo, cos, sin):
    mskh, swph = _mask_tables()
    idh = np.eye(128, dtype=np.float32)
    xTh = np.ascontiguousarray(x.reshape(T, DIM).T.astype(np.float16))
    cosTh = np.ascontiguousarray(cos.T)
    # rotate_half sign fold: out = u*cos + u_rot*sin_signed
    sinTh = np.ascontiguousarray(sin.T).copy()
    sinTh[: HD // 2, :] *= -1.0
    in_maps = []
    for c in range(NCORES):
        g = c // 2
        # wo slice: [DIM, HPC*HD] -> [128, HPC*DIM] with [p, h*DIM+n] layout
        woc = wo[:, c * HPC * HD:(c + 1) * HPC * HD]  # [DIM, 256]
        woh = np.ascontiguousarray(
            woc.T.reshape(HPC, 128, DIM).transpose(1, 0, 2)
            .reshape(128, HPC * DIM)).astype(np.float16)
        in_maps.append({
            "xT": xTh,
            "wqT": _sbufify(wq[c * HPC * HD:(c + 1) * HPC * HD, :]),
            "wkT": _sbufify(wk[g * HD:(g + 1) * HD, :]),
            "wvT": _sbufify(wv[g * HD:(g + 1) * HD, :]),
            "woT": woh,
            "cosT": cosTh.astype(np.float16),
            "sinT": sinTh.astype(np.float16),
            "swpT": swph,
            "idT": idh,
            "mskT": mskh,
        })
    return in_maps


def _get_exec(nrepeat=1):
    """Build (once) a cached jitted SPMD executable over the 8 cores."""
    key = ("exec", nrepeat)
    if key in _CACHE:
        return _CACHE[key]

    import jax
    from jax.sharding import Mesh, PartitionSpec
    from jax.experimental.shard_map import shard_map
    from concourse import bass2jax
    import concourse.mybir as mybir

    nc = _build_nc(nrepeat=nrepeat)

    bass2jax.install_neuronx_cc_hook()

    part_name = (nc.partition_id_tensor.name
                 if nc.partition_id_tensor else None)
    in_names, out_names, out_avals = [], [], []
    for alloc in nc.m.functions[0].allocations:
        if not isinstance(alloc, mybir.MemoryLocationSet):
            continue
        name = alloc.memorylocations[0].name
        if alloc.kind == "ExternalInput":
            if name != part_name:
                in_names.append(name)
        elif alloc.kind == "ExternalOutput":
            out_names.append(name)
            out_avals.append(jax.core.ShapedArray(
                tuple(alloc.tensor_shape), mybir.dt.np(alloc.dtype)))

    bind_names = in_names + out_names
    if part_name is not None:
        bind_names = bind_names + [part_name]

    def _body(*args):
        operands = list(args)
        if part_name is not None:
            operands.append(bass2jax.partition_id_tensor())
        outs = bass2jax._bass_exec_p.bind(
            *operands,
            out_avals=tuple(out_avals),
            in_names=tuple(bind_names),
            out_names=tuple(out_names),
            lowering_input_output_aliases=(),
            sim_require_finite=True,
            sim_require_nnan=True,
            nc=nc,
        )
        return tuple(outs)

    devices = jax.devices()[:NCORES]
    mesh = Mesh(np.asarray(devices), ("core",))
    n_in = len(in_names)
    n_out = len(out_names)
    sharded = jax.jit(
        shard_map(
            _body, mesh=mesh,
            in_specs=(PartitionSpec("core"),) * (n_in + n_out),
            out_specs=(PartitionSpec("core"),) * n_out,
            check_rep=False,
        ),
        keep_unused=True,
    )
    _CACHE[key] = (sharded, in_names, out_names, out_avals, mesh)
    return _CACHE[key]


def _concat_inputs(in_maps, in_names):
    return [
        np.concatenate([in_maps[c][name] for c in range(NCORES)], axis=0)
        for name in in_names
    ]


def _zero_outs(out_avals):
    return [
        np.zeros((NCORES * a.shape[0], *a.shape[1:]), a.dtype)
        for a in out_avals
    ]


def kernel(**inputs):
    sharded, in_names, out_names, out_avals, _ = _get_exec()

    in_maps = _shard_inputs(
        np.asarray(inputs["x"], dtype=np.float32),
        np.asarray(inputs["wq"], dtype=np.float32),
        np.asarray(inputs["wk"], dtype=np.float32),
        np.asarray(inputs["wv"], dtype=np.float32),
        np.asarray(inputs["wo"], dtype=np.float32),
        np.asarray(inputs["cos"], dtype=np.float32),
        np.asarray(inputs["sin"], dtype=np.float32),
    )
    concat_in = _concat_inputs(in_maps, in_names)
    out_arrs = sharded(*concat_in, *_zero_outs(out_avals))

    full = np.asarray(out_arrs[out_names.index("out")])
    acc = full.reshape(NCORES, T, DIM).astype(np.float32).sum(axis=0)
    return acc.reshape(1, T, DIM)


# revision 21
# speedup vs baseline: 415.5118x; 1.0385x over previous
"""GQA attention forward (B=1, T=2048, DIM=2048, H=16, KV=4, HD=128) on 8 trn2 cores.

Sharding: tensor-parallel over heads. Core c owns q-heads {2c, 2c+1} and kv-head
c//2 (kv work duplicated across the pair of cores sharing it).

Kernel structure (per core), quarter-major pipeline over four 512-token chunks:
  per quarter tq: qkv projections (fp16 matmuls, f32 PSUM), RoPE on q/k with the
  rotate-half partition swap done as a PE matmul against a host-supplied
  permutation matrix (sign folded into the sin table), v PE-transposed to
  natural [t, hd] fp16 layout; then per q-head: causal scores S^T[k, q] (f32r,
  diagonal blocks trimmed to the valid q-range, min N=256), causal mask applied
  as a matmul-accumulated additive bias (only 4 distinct mask matrices exist,
  host-supplied), exp on ACT with 1/sqrt(hd) folded into the activation scale
  (fp16 out), softmax denominators via ones-matmul accumulation in PSUM, A^T
  accumulated in PSUM over k-blocks, normalized by DVE reciprocal+mul into fp16
  aT; wo matmuls (fp16) interleaved one quarter behind so output DMA streams
  during attention.

All DMAs use >=2KB-per-partition lines: weights/masks are pre-rearranged on the
host into their SBUF layouts (contiguous loads), x is loaded as [128,1024] fp16
half-tiles, out is staged and written as [128,1024] fp16 pairs.

Host: sums the 8 partial [T, DIM] fp16 outputs in f32.

_build_nc(nrepeat=N) wraps the whole body in an on-device For_i loop (used by
test.py to measure per-iteration HW exec time without dispatch overhead).
"""

import sys

if "/opt/trn_rl_repo" not in sys.path:
    sys.path.insert(0, "/opt/trn_rl_repo")

import numpy as np

T = 2048
DIM = 2048
H = 16
KV = 4
HD = 128
NCORES = 8
HPC = H // NCORES            # q heads per core = 2
SCALE = float(HD) ** -0.5
ND = DIM // 128              # dim chunks = 16
NT = T // 128                # t blocks = 16
NQC = T // 512               # q 512-chunks = 4

# causal diagonal-block trim: for diag offset j, start q at QOFF[j] (N>=256)
QOFF = [0, 128, 256, 384]

_CACHE = {}


def _build_nc(nrepeat=1):
    from contextlib import ExitStack, nullcontext

    from concourse import bacc
    import concourse.mybir as mybir
    import concourse.tile as tile
    from concourse.masks import make_identity

    f32 = mybir.dt.float32
    f32r = mybir.dt.float32r
    f16 = mybir.dt.float16
    Exp = mybir.ActivationFunctionType.Exp

    nc = bacc.Bacc("TRN2", target_bir_lowering=False, debug=False,
                   enable_asserts=False)

    # all weight-like inputs arrive pre-rearranged into their SBUF layouts
    xT = nc.dram_tensor("xT", [DIM, T], f16, kind="ExternalInput").ap()
    wqT = nc.dram_tensor("wqT", [128, ND * HPC * HD], f16,
                         kind="ExternalInput").ap()
    wkT = nc.dram_tensor("wkT", [128, ND * HD], f16, kind="ExternalInput").ap()
    wvT = nc.dram_tensor("wvT", [128, ND * HD], f16, kind="ExternalInput").ap()
    woT = nc.dram_tensor("woT", [128, HPC * DIM], f16,
                         kind="ExternalInput").ap()
    cosT = nc.dram_tensor("cosT", [HD, T], f16, kind="ExternalInput").ap()
    sinT = nc.dram_tensor("sinT", [HD, T], f16, kind="ExternalInput").ap()
    swpT = nc.dram_tensor("swpT", [128, 128], f16, kind="ExternalInput").ap()
    mskT = nc.dram_tensor("mskT", [128, 4 * 512], f16,
                          kind="ExternalInput").ap()
    out = nc.dram_tensor("out", [T, DIM], f16, kind="ExternalOutput").ap()

    with tile.TileContext(nc) as tc, \
         (tc.For_i(0, nrepeat) if nrepeat > 1 else nullcontext()), \
         ExitStack() as ctx:
        const = ctx.enter_context(tc.tile_pool(name="const", bufs=1))
        wpool = ctx.enter_context(tc.tile_pool(name="wts", bufs=1))
        qkv = ctx.enter_context(tc.tile_pool(name="qkv", bufs=1))
        apool = ctx.enter_context(tc.tile_pool(name="Apool", bufs=1))
        xpool = ctx.enter_context(tc.tile_pool(name="xp", bufs=16))
        rp = ctx.enter_context(tc.tile_pool(name="rope", bufs=2))
        ppool = ctx.enter_context(tc.tile_pool(name="pp", bufs=32))
        rpool = ctx.enter_context(tc.tile_pool(name="rcp", bufs=2))
        ostage = ctx.enter_context(tc.tile_pool(name="ost", bufs=4))
        ps1 = ctx.enter_context(tc.tile_pool(name="p1ps", bufs=3, space="PSUM"))
        sps = ctx.enter_context(tc.tile_pool(name="sps", bufs=2, space="PSUM"))
        otp = ctx.enter_context(tc.tile_pool(name="otp", bufs=1, space="PSUM"))
        wops = ctx.enter_context(tc.tile_pool(name="wops", bufs=2, space="PSUM"))

        ident = const.tile([128, 128], f16)
        make_identity(nc, ident)
        ones16 = const.tile([128, 128], f16)
        nc.vector.memset(ones16, 1.0)

        qT_s = qkv.tile([128, HPC * T], f16)   # rope'd q, [hd, t] per head
        kT_s = qkv.tile([128, T], f16)         # rope'd k, [hd, t]
        vT_s = qkv.tile([128, T], f16)          # v, [hd, t]
        v_s = qkv.tile([128, NT * HD], f16)     # v natural [t%128, hd] per blk
        aT_s = [apool.tile([128, T], f16, name=f"aT{h}") for h in range(HPC)]

        def load_x_tiles(c0, cols, lo, hi, xts, tag):
            nb = 32 if tag == "xq" else 16
            for d in range(lo, hi):
                xt = xpool.tile([128, cols], f16, tag=tag, bufs=nb,
                                name=f"xt{c0}_{d}")
                nc.sync.dma_start(
                    xt, xT[d * 128:(d + 1) * 128, c0:c0 + cols])
                xts.append(xt)

        # quarter 0 narrow (startup-critical), middle half wide, quarter 3
        # narrow — minimizes bytes gating the first projections
        wk_s = wpool.tile([128, ND * HD], f16)
        nc.sync.dma_start(wk_s, wkT)
        xq0 = []
        load_x_tiles(0, 512, 0, 8, xq0, "xq")
        wq_s = wpool.tile([128, ND * HPC * HD], f16)
        nc.sync.dma_start(wq_s, wqT)
        load_x_tiles(0, 512, 8, 12, xq0, "xq")
        cos_s = const.tile([128, T], f16)
        nc.sync.dma_start(cos_s, cosT)
        sin_s = const.tile([128, T], f16)
        nc.sync.dma_start(sin_s, sinT)
        swp_s = const.tile([128, 128], f16)
        nc.sync.dma_start(swp_s, swpT)
        load_x_tiles(0, 512, 12, 16, xq0, "xq")
        wv_s = wpool.tile([128, ND * HD], f16)
        nc.sync.dma_start(wv_s, wvT)
        msk_s = const.tile([128, 4 * 512], f16)
        nc.sync.dma_start(msk_s, mskT)
        xmid = []
        load_x_tiles(512, 1024, 0, 16, xmid, "xt")
        wo_s = wpool.tile([128, HPC * DIM], f16)
        nc.sync.dma_start(wo_s, woT)
        xq3 = []
        load_x_tiles(1536, 512, 0, 16, xq3, "xq")

        def xts_for(tq):
            if tq == 0:
                return xq0
            if tq == 3:
                return xq3
            c0 = (tq - 1) * 512
            return [t[:, c0:c0 + 512] for t in xmid]

        def rope(u, c0, t0, cols=512):
            # u[:, c0:c0+cols] <- u*cos + rotate_half(u)*sin  (sin sign-folded)
            us = u[:, c0:c0 + cols]
            cs = cos_s[:, t0:t0 + cols]
            ss = sin_s[:, t0:t0 + cols]
            tmp = rp.tile([128, cols], f32, tag="rtmp")
            rot_ps = ps1.tile([128, cols], f32, tag="pps", name="rotp")
            # partition swap as a PE matmul against a permutation matrix
            nc.tensor.matmul(rot_ps, swp_s, us, start=True, stop=True)
            rot = rp.tile([128, cols], f32, tag="rot")
            nc.vector.tensor_mul(tmp, us, cs)
            nc.vector.tensor_mul(rot, rot_ps, ss)
            nc.vector.tensor_add(us, tmp, rot)

        def proj(acc_tag, w_ap, xts, dst, c0):
            acc = ps1.tile([128, 512], f32, tag="pps", name=acc_tag)
            for d in range(ND):
                nc.tensor.matmul(acc, w_ap(d), xts[d],
                                 start=(d == 0), stop=(d == ND - 1))
            nc.scalar.copy(dst[:, c0:c0 + 512], acc)

        def quarter_proj(tq):
            xts = xts_for(tq)
            t0 = tq * 512
            proj(f"k{tq}", lambda d: wk_s[:, d * HD:(d + 1) * HD],
                 xts, kT_s, t0)
            proj(f"q0_{tq}",
                 lambda d: wq_s[:, d * HPC * HD:d * HPC * HD + HD],
                 xts, qT_s, t0)
            rope(kT_s, t0, t0)
            proj(f"q1_{tq}",
                 lambda d: wq_s[:, d * HPC * HD + HD:(d + 1) * HPC * HD],
                 xts, qT_s, T + t0)
            rope(qT_s, t0, t0)
            proj(f"v{tq}", lambda d: wv_s[:, d * HD:(d + 1) * HD],
                 xts, vT_s, t0)
            rope(qT_s, T + t0, t0)
            for tb in range(tq * 4, tq * 4 + 4):
                vt = ps1.tile([128, 128], f16, tag="pps", name=f"vt{tb}")
                nc.tensor.transpose(
                    vt, vT_s[:, tb * 128:(tb + 1) * 128], ident)
                nc.scalar.copy(v_s[:, tb * HD:(tb + 1) * HD], vt)

        def scores_step(h, qc, kb, ptiles):
            qTh = qT_s[:, h * T:(h + 1) * T]
            j = kb - 4 * qc
            qo = QOFF[j] if j >= 0 else 0
            n = 512 - qo
            s_ps = sps.tile([128, n], f32, tag="s", name=f"s{h}{qc}_{kb}")
            nc.tensor.matmul(
                s_ps, kT_s[:, kb * 128:(kb + 1) * 128],
                qTh[:, qc * 512 + qo:(qc + 1) * 512],
                start=True, stop=(j < 0))
            if j >= 0:
                # causal mask as an accumulated additive bias matmul
                nc.tensor.matmul(
                    s_ps, ident, msk_s[:, j * 512:j * 512 + n],
                    start=False, stop=True)
            p_sb = ppool.tile([128, n], f16, tag="p", name=f"p{h}{qc}_{kb}")
            nc.scalar.activation(p_sb, s_ps, Exp, scale=SCALE)
            ptiles.append((p_sb, qo))

        def av_fin(h, qc, ptiles, oT):
            nkb = 4 * qc + 4
            # dn shares ps1's "pps" slots (proj accs and dn alternate in time)
            dn = ps1.tile([128, 512], f32, tag="pps", name=f"dn{h}_{qc}")
            for kb in range(nkb):
                p_sb, qo = ptiles[kb]
                nc.tensor.matmul(
                    dn[:, qo:], ones16, p_sb,
                    start=(kb == 0), stop=(kb == nkb - 1))
            rec = rpool.tile([128, 512], f32, tag="rec")
            nc.vector.reciprocal(rec, dn)
            nc.vector.tensor_mul(
                aT_s[h][:, qc * 512:(qc + 1) * 512], oT, rec)

        def wo_pair(qc, tb, n2):
            ops = []
            for k in range(2):
                op = wops.tile([128, 512], f32, tag="op",
                               name=f"op{tb}_{n2}_{k}")
                for h in range(HPC):
                    nc.tensor.matmul(
                        op, aT_s[h][:, tb * 128:(tb + 1) * 128],
                        wo_s[:, h * DIM + (n2 * 2 + k) * 512:
                             h * DIM + (n2 * 2 + k + 1) * 512],
                        start=(h == 0), stop=(h == HPC - 1))
                ops.append(op)
            ob = ostage.tile([128, 1024], f16, tag="ob")
            nc.vector.tensor_copy(ob[:, 0:512], ops[0])
            nc.vector.tensor_copy(ob[:, 512:1024], ops[1])
            nc.sync.dma_start(
                out[tb * 128:(tb + 1) * 128,
                    n2 * 1024:(n2 + 1) * 1024], ob)

        for tq in range(4):
            quarter_proj(tq)
            nkb = 4 * tq + 4
            # phase A: scores h0 interleaved with previous quarter's wo pairs
            # (keeps PE fed while ACT drains the exp of each score tile)
            wo_steps = ([(tq - 1, tb, n2)
                         for tb in range((tq - 1) * 4, tq * 4)
                         for n2 in range(2)] if tq > 0 else [])
            pt0 = []
            for i in range(max(nkb, len(wo_steps))):
                if i < nkb:
                    scores_step(0, tq, i, pt0)
                if i < len(wo_steps):
                    wo_pair(*wo_steps[i])
            # phase B: scores h1 interleaved with h0's AV accumulation
            pt1 = []
            oT0 = otp.tile([128, 512], f32, tag="oT", name=f"oT0_{tq}")
            for i in range(nkb):
                scores_step(1, tq, i, pt1)
                p_sb, qo = pt0[i]
                nc.tensor.matmul(
                    oT0[:, qo:], v_s[:, i * HD:(i + 1) * HD], p_sb,
                    start=(i == 0), stop=(i == nkb - 1))
            av_fin(0, tq, pt0, oT0)
            # phase C: h1's AV + dn
            oT1 = otp.tile([128, 512], f32, tag="oT", name=f"oT1_{tq}")
            for i in range(nkb):
                p_sb, qo = pt1[i]
                nc.tensor.matmul(
                    oT1[:, qo:], v_s[:, i * HD:(i + 1) * HD], p_sb,
                    start=(i == 0), stop=(i == nkb - 1))
            av_fin(1, tq, pt1, oT1)
        for tb in range(12, 16):
            for n2 in range(2):
                wo_pair(3, tb, n2)

    nc.compile()
    return nc


def _mask_tables():
    # M_j[p, qf] = 0 where (QOFF[j] + qf) - 128*j - p >= 0 else -30000
    msk = np.zeros((4, 128, 512), np.float32)
    p = np.arange(128)[:, None]
    qf = np.arange(512)[None, :]
    for j in range(4):
        cond = (QOFF[j] + qf - 128 * j - p) >= 0
        msk[j] = np.where(cond, 0.0, -30000.0)
    # [p, j*512+qf] layout
    mskh = np.ascontiguousarray(msk.transpose(1, 0, 2).reshape(128, 4 * 512))
    swp = np.zeros((128, 128), np.float32)
    # rot = swp.T @ u : rot[i] = u[i+64] (i<64), rot[i] = u[i-64] (i>=64)
    for i in range(64):
        swp[i + 64, i] = 1.0
        swp[i, i + 64] = 1.0
    return mskh, swp


def _sbufify(w):
    """[rows=outdim, DIM] weight -> pre-rearranged [128, ND*rows] fp16 with the
    d-chunk-major SBUF layout: dst[p, d*rows + n] = w[n, d*128 + p]."""
    rows = w.shape[0]
    a = np.ascontiguousarray(
        w.T.reshape(ND, 128, rows).transpose(1, 0, 2).reshape(128, ND * rows))
    return a.astype(np.float16)


def _shard_inputs(x, wq, wk, wv, wo, cos, sin):
    mskh, swph = _mask_tables()
    xTh = np.ascontiguousarray(x.reshape(T, DIM).T.astype(np.float16))
    cosTh = np.ascontiguousarray(cos.T)
    # rotate_half sign fold: out = u*cos + u_rot*sin_signed
    sinTh = np.ascontiguousarray(sin.T).copy()
    sinTh[: HD // 2, :] *= -1.0
    in_maps = []
    for c in range(NCORES):
        g = c // 2
        # wo slice: [DIM, HPC*HD] -> [128, HPC*DIM] with [p, h*DIM+n] layout
        woc = wo[:, c * HPC * HD:(c + 1) * HPC * HD]  # [DIM, 256]
        woh = np.ascontiguousarray(
            woc.T.reshape(HPC, 128, DIM).transpose(1, 0, 2)
            .reshape(128, HPC * DIM)).astype(np.float16)
        in_maps.append({
            "xT": xTh,
            "wqT": _sbufify(wq[c * HPC * HD:(c + 1) * HPC * HD, :]),
            "wkT": _sbufify(wk[g * HD:(g + 1) * HD, :]),
            "wvT": _sbufify(wv[g * HD:(g + 1) * HD, :]),
            "woT": woh,
            "cosT": cosTh.astype(np.float16),
            "sinT": sinTh.astype(np.float16),
            "swpT": swph.astype(np.float16),
            "mskT": mskh.astype(np.float16),
        })
    return in_maps


def _get_exec(nrepeat=1):
    """Build (once) a cached jitted SPMD executable over the 8 cores."""
    key = ("exec", nrepeat)
    if key in _CACHE:
        return _CACHE[key]

    import jax
    from jax.sharding import Mesh, PartitionSpec
    from jax.experimental.shard_map import shard_map
    from concourse import bass2jax
    import concourse.mybir as mybir

    nc = _build_nc(nrepeat=nrepeat)

    bass2jax.install_neuronx_cc_hook()

    part_name = (nc.partition_id_tensor.name
                 if nc.partition_id_tensor else None)
    in_names, out_names, out_avals = [], [], []
    for alloc in nc.m.functions[0].allocations:
        if not isinstance(alloc, mybir.MemoryLocationSet):
            continue
        name = alloc.memorylocations[0].name
        if alloc.kind == "ExternalInput":
            if name != part_name:
                in_names.append(name)
        elif alloc.kind == "ExternalOutput":
            out_names.append(name)
            out_avals.append(jax.core.ShapedArray(
                tuple(alloc.tensor_shape), mybir.dt.np(alloc.dtype)))

    bind_names = in_names + out_names
    if part_name is not None:
        bind_names = bind_names + [part_name]

    def _body(*args):
        operands = list(args)
        if part_name is not None:
            operands.append(bass2jax.partition_id_tensor())
        outs = bass2jax._bass_exec_p.bind(
            *operands,
            out_avals=tuple(out_avals),
            in_names=tuple(bind_names),
            out_names=tuple(out_names),
            lowering_input_output_aliases=(),
            sim_require_finite=True,
            sim_require_nnan=True,
            nc=nc,
        )
        return tuple(outs)

    devices = jax.devices()[:NCORES]
    mesh = Mesh(np.asarray(devices), ("core",))
    n_in = len(in_names)
    n_out = len(out_names)
    sharded = jax.jit(
        shard_map(
            _body, mesh=mesh,
            in_specs=(PartitionSpec("core"),) * (n_in + n_out),
            out_specs=(PartitionSpec("core"),) * n_out,
            check_rep=False,
        ),
        keep_unused=True,
    )
    _CACHE[key] = (sharded, in_names, out_names, out_avals, mesh)
    return _CACHE[key]


def _concat_inputs(in_maps, in_names):
    return [
        np.concatenate([in_maps[c][name] for c in range(NCORES)], axis=0)
        for name in in_names
    ]


def _zero_outs(out_avals):
    return [
        np.zeros((NCORES * a.shape[0], *a.shape[1:]), a.dtype)
        for a in out_avals
    ]


def kernel(**inputs):
    sharded, in_names, out_names, out_avals, _ = _get_exec()

    in_maps = _shard_inputs(
        np.asarray(inputs["x"], dtype=np.float32),
        np.asarray(inputs["wq"], dtype=np.float32),
        np.asarray(inputs["wk"], dtype=np.float32),
        np.asarray(inputs["wv"], dtype=np.float32),
        np.asarray(inputs["wo"], dtype=np.float32),
        np.asarray(inputs["cos"], dtype=np.float32),
        np.asarray(inputs["sin"], dtype=np.float32),
    )
    concat_in = _concat_inputs(in_maps, in_names)
    out_arrs = sharded(*concat_in, *_zero_outs(out_avals))

    full = np.asarray(out_arrs[out_names.index("out")])
    acc = full.reshape(NCORES, T, DIM).astype(np.float32).sum(axis=0)
    return acc.reshape(1, T, DIM)
